# revision 1
# baseline (speedup 1.0000x reference)
"""Two-layer Keras-style GRU (reset_after=True, sigmoid/relu) + dense head
on 8 Trainium2 NeuronCores, data-parallel over batch.

Full shapes: x[128,512,128]; GRU1 F=128->U=512; GRU2 512->512; head 512->1.
Each core handles 16 batch rows end-to-end; no collectives.

Layout (per core, mapping "A"): recurrence matmul out = h @ U with
lhsT = h.T tiles [128,16] (stationary, rebuilt per step via PE transpose)
and U streaming as the f32r moving operand. Input projections are bulk
matmuls into HBM scratch; per-step gx[t] is injected into PSUM with a
16x16-identity matmul so PE (not DVE) pays for the add.
"""
import os
import sys
import time

sys.path.insert(0, "/opt/trn_rl_repo")
sys.path.insert(0, "/opt/trn_rl_repo/concourse")

import numpy as np

import concourse.bass as bass
import concourse.bacc as bacc
import concourse.tile as tile
from concourse import mybir
from concourse.bass_utils import run_bass_kernel_spmd
from concourse.masks import make_identity

F32 = mybir.dt.float32
F32R = mybir.dt.float32r

N_CORES = 8
B_TOT, T_FULL, F_IN, U = 128, 512, 128, 512
B = B_TOT // N_CORES          # 16 local batch
G3 = 3 * U                    # 1536 gate cols


def _emit_gru_scan(nc, tc, ctx, *, T, gx_view, U_sb, brh_row, h1T_view,
                   id_pad, id16f, name):
    """Emit one GRU scan over T steps.

    gx_view: DRAM AP view [T, B, G3] fp32 — per-step slice [B, G3]
             already contains x@W + bi (+ br for z,r columns).
    U_sb:    SBUF [128, 4, G3] f32r recurrent weights.
    brh_row: (ones[1,B], row[1,U]) f32r recurrent h-gate bias, or None.
    h1T_view: DRAM AP view [T, 128, KT, B] f32r to receive h.T per step, or None.
    Returns the final hT sbuf tile [128, 4*B] f32r.
    """
    KT = U // 128  # 4 k-tiles

    gx_pool = ctx.enter_context(tc.tile_pool(name=f"{name}_gx", bufs=4))
    state_pool = ctx.enter_context(tc.tile_pool(name=f"{name}_state", bufs=2))
    hT_pool = ctx.enter_context(tc.tile_pool(name=f"{name}_hT", bufs=2))
    gate_pool = ctx.enter_context(tc.tile_pool(name=f"{name}_gates", bufs=2))
    ps_zr = ctx.enter_context(tc.tile_pool(name=f"{name}_ps_zr", bufs=2, space="PSUM"))
    ps_rh = ctx.enter_context(tc.tile_pool(name=f"{name}_ps_rh", bufs=1, space="PSUM"))
    ps_tr = ctx.enter_context(tc.tile_pool(name=f"{name}_ps_tr", bufs=2, space="PSUM"))

    # initial state: zeros.  hT is padded to 128 free cols per k-tile
    # (f32r matmuls need col_grp=0xf, i.e. full-128 output partitions);
    # cols B..127 stay zero so output rows B..127 are zero/garbage-free.
    h = state_pool.tile([B, U], F32, tag="h")
    nc.vector.memset(h, 0.0)
    zeros_f = state_pool.tile([128, KT, 128], F32, tag="zf")
    nc.vector.memset(zeros_f, 0.0)
    hT_a = hT_pool.tile([128, KT, 128], F32R, tag="hT")
    hT_b = hT_pool.tile([128, KT, 128], F32R, tag="hT")
    hT_bufs = [hT_a, hT_b]
    for b_ in hT_bufs:
        nc.vector.tensor_copy(b_[:], zeros_f[:])
    hT = hT_bufs[1]

    for t in range(T):
        gx_t = gx_pool.tile([B, G3], F32R, tag="gx")
        nc.sync.dma_start(gx_t[:], gx_view[t, :, :].bitcast(F32R))

        # --- PE: rec = gx[t] + h @ U  (z,r slices), rec_h = h @ U_h (+br_h)
        pz = []
        for n in range(2):  # z, r 512-col slices
            p = ps_zr.tile([128, 512], F32, tag=f"zr{n}")
            nc.tensor.matmul(p[:], id_pad[:], gx_t[:, n * 512:(n + 1) * 512],
                             start=True, stop=False)
            for k in range(KT):
                nc.tensor.matmul(p[:], hT[:, k, :],
                                 U_sb[:, k, n * 512:(n + 1) * 512],
                                 start=False, stop=(k == KT - 1))
            pz.append(p)
        prh = ps_rh.tile([128, 512], F32, tag="rh")
        first = True
        for k in range(KT):
            nc.tensor.matmul(prh[:], hT[:, k, :],
                             U_sb[:, k, 1024:1536],
                             start=first, stop=(k == KT - 1 and brh_row is None))
            first = False
        if brh_row is not None:
            ones_pad, brh = brh_row
            nc.tensor.matmul(prh[:], ones_pad[:], brh[:], start=False, stop=True)

        # --- gates (rows 0:B of the padded psum tiles)
        z = gate_pool.tile([B, 512], F32, tag="z")
        r = gate_pool.tile([B, 512], F32, tag="r")
        nc.scalar.activation(z[:], pz[0][0:B, :],
                             mybir.ActivationFunctionType.Sigmoid)
        nc.scalar.activation(r[:], pz[1][0:B, :],
                             mybir.ActivationFunctionType.Sigmoid)
        t1 = gate_pool.tile([B, 512], F32, tag="t1")
        nc.vector.tensor_mul(t1[:], r[:], prh[0:B, :])        # r * rec_h
        t2 = gate_pool.tile([B, 512], F32, tag="t2")
        nc.vector.tensor_add(t2[:], t1[:], gx_t[:, 1024:1536].bitcast(F32))
        hh = gate_pool.tile([B, 512], F32, tag="hh")
        nc.scalar.activation(hh[:], t2[:], mybir.ActivationFunctionType.Relu)
        # h_new = hh + z*(h - hh)
        d = gate_pool.tile([B, 512], F32, tag="d")
        nc.vector.scalar_tensor_tensor(d[:], hh[:], -1.0, h[:],
                                       mybir.AluOpType.mult, mybir.AluOpType.add)
        e = gate_pool.tile([B, 512], F32, tag="e")
        nc.vector.tensor_mul(e[:], z[:], d[:])
        h = state_pool.tile([B, U], F32, tag="h")
        nc.vector.tensor_add(h[:], e[:], hh[:])

        # --- transpose h -> hT for next step (and optional HBM sink)
        ptr = ps_tr.tile([128, KT * B], F32, tag="tr")
        for k in range(KT):
            nc.tensor.transpose(ptr[:, k * B:(k + 1) * B],
                                h[:, k * 128:(k + 1) * 128], id16f[:])
        hT = hT_bufs[t % 2]
        nc.vector.tensor_copy(
            hT[:, :, 0:B], ptr[:].rearrange("p (k b) -> p k b", k=KT))
        if h1T_view is not None:
            nc.sync.dma_start(h1T_view[t, :, :, :], hT[:, :, 0:B])

    return h


def build_bass(T=T_FULL, with_bi1=False, with_br1=False, with_bi2=False,
               with_br2=False):
    nc = bacc.Bacc("TRN2", target_bir_lowering=False, debug=False,
                   enable_asserts=False, num_devices=N_CORES)

    x_d = nc.dram_tensor("x", [B, T, F_IN], F32, kind="ExternalInput").ap()
    W1_d = nc.dram_tensor("W1", [F_IN, G3], F32, kind="ExternalInput").ap()
    U1_d = nc.dram_tensor("U1", [U, G3], F32, kind="ExternalInput").ap()
    bi1_d = nc.dram_tensor("bi1", [G3], F32, kind="ExternalInput").ap()
    br1_d = nc.dram_tensor("br1", [G3], F32, kind="ExternalInput").ap()
    W2_d = nc.dram_tensor("W2", [U, G3], F32, kind="ExternalInput").ap()
    U2_d = nc.dram_tensor("U2", [U, G3], F32, kind="ExternalInput").ap()
    bi2_d = nc.dram_tensor("bi2", [G3], F32, kind="ExternalInput").ap()
    br2_d = nc.dram_tensor("br2", [G3], F32, kind="ExternalInput").ap()
    Wd_d = nc.dram_tensor("Wd", [U, 1], F32, kind="ExternalInput").ap()
    bd_d = nc.dram_tensor("bd", [1], F32, kind="ExternalInput").ap()
    out_d = nc.dram_tensor("out", [B, 1], F32, kind="ExternalOutput").ap()

    BT = B * T
    MT = BT // 128            # number of 128-row bt tiles
    KT = U // 128

    with tile.TileContext(nc) as tc:
        from contextlib import ExitStack
        with ExitStack() as ctx:
            const = ctx.enter_context(tc.tile_pool(name="const", bufs=1))
            dram = ctx.enter_context(tc.tile_pool(name="dram", bufs=1, space="DRAM"))

            # ---- constants / weights to SBUF
            # id16f: fp32 identity for PE transposes.  id_pad: [B,128] f32r
            # zero-padded identity (stationary of the gx-inject matmul; padded
            # so f32r matmul output spans all 128 partitions).
            id16f = const.tile([B, B], F32)
            make_identity(nc, id16f)
            id128 = const.tile([128, 128], F32)
            make_identity(nc, id128)
            id_pad_f = const.tile([B, 128], F32)
            nc.vector.memset(id_pad_f, 0.0)
            make_identity(nc, id_pad_f[:, 0:B], nomemset=True)
            id_pad = const.tile([B, 128], F32R)
            nc.vector.tensor_copy(id_pad[:], id_pad_f[:])

            W1_sb = const.tile([128, G3], F32R)
            nc.sync.dma_start(W1_sb[:], W1_d[:].bitcast(F32R))
            U1_sb = const.tile([128, KT, G3], F32R)
            nc.sync.dma_start(U1_sb[:],
                              U1_d.rearrange("(a p) g -> p a g", p=128).bitcast(F32R))
            W2_sb = const.tile([128, KT, G3], F32R)
            nc.sync.dma_start(W2_sb[:],
                              W2_d.rearrange("(a p) g -> p a g", p=128).bitcast(F32R))
            U2_sb = const.tile([128, KT, G3], F32R)
            nc.sync.dma_start(U2_sb[:],
                              U2_d.rearrange("(a p) g -> p a g", p=128).bitcast(F32R))
            Wd_sb = const.tile([128, KT, 1], F32R)
            nc.sync.dma_start(Wd_sb[:],
                              Wd_d.rearrange("(a p) o -> p a o", p=128).bitcast(F32R))
            bd_sb = const.tile([B, 1], F32)
            nc.sync.dma_start(bd_sb[:], bd_d.to_broadcast((B, 1)))

            # optional bias rows (only emitted when nonzero)
            ones1x128 = None
            ones1xB = None
            bias1_row = None   # [1, G3] = bi1 (+ br1 on z,r cols)
            bias2_row = None
            brh1 = None
            brh2 = None
            def _bias_row(bi_dram, br_dram, with_bi, with_br, tag):
                if not (with_bi or with_br):
                    return None, None
                row = const.tile([1, G3], F32R, tag=f"biasrow_{tag}")
                if with_bi:
                    nc.sync.dma_start(row[:], bi_dram[None, :].bitcast(F32R))
                else:
                    zrow = const.tile([1, G3], F32, tag=f"zrow_{tag}")
                    nc.vector.memset(zrow, 0.0)
                    nc.vector.tensor_copy(row[:], zrow[:])
                brh_t = None
                if with_br:
                    tmp = const.tile([1, 1024], F32, tag=f"biastmp_{tag}")
                    nc.sync.dma_start(tmp[:], br_dram[None, 0:1024])
                    nc.vector.tensor_add(row[:, 0:1024],
                                         row[:, 0:1024].bitcast(F32), tmp[:])
                    brh_t = const.tile([1, U], F32R, tag=f"brh_{tag}")
                    nc.sync.dma_start(brh_t[:],
                                      br_dram[None, 1024:1536].bitcast(F32R))
                return row, brh_t

            bias1_row, brh1_t = _bias_row(bi1_d, br1_d, with_bi1, with_br1, "1")
            bias2_row, brh2_t = _bias_row(bi2_d, br2_d, with_bi2, with_br2, "2")
            if with_br1 or with_br2 or with_bi1 or with_bi2:
                ones_f = const.tile([1, 128], F32)
                nc.vector.memset(ones_f, 1.0)
                ones1x128 = const.tile([1, 128], F32R)
                nc.vector.tensor_copy(ones1x128[:], ones_f[:])
            if with_br1:
                brh1 = (ones1x128, brh1_t)
            if with_br2:
                brh2 = (ones1x128, brh2_t)

            # ---- DRAM scratch
            gx1_hbm = dram.tile([BT, G3], F32)
            gx2_hbm = dram.tile([BT, G3], F32)
            h1T_hbm = dram.tile([U, BT], F32R)

            # ---- phase X+G1: xT tiles and gx1 = x @ W1 (+bias row)
            x_flat = x_d.flatten_outer_dims()   # [BT, F_IN]
            with tc.tile_pool(name="xin", bufs=3) as xin_pool, \
                 tc.tile_pool(name="xT", bufs=3) as xT_pool, \
                 tc.tile_pool(name="ps_xt", bufs=2, space="PSUM") as ps_xt, \
                 tc.tile_pool(name="ps_g", bufs=4, space="PSUM") as ps_g:
                for m in range(MT):
                    xin = xin_pool.tile([128, F_IN], F32, tag="xin")
                    nc.sync.dma_start(xin[:], x_flat[m * 128:(m + 1) * 128, :])
                    pxt = ps_xt.tile([128, 128], F32, tag="pxt")
                    nc.tensor.transpose(pxt[:], xin[:], id128[:])
                    xT = xT_pool.tile([128, 128], F32R, tag="xT")
                    nc.vector.tensor_copy(xT[:], pxt[:].bitcast(F32R))
                    for n in range(3):
                        pg = ps_g.tile([128, 512], F32, tag="pg")
                        nc.tensor.matmul(pg[:], xT[:], W1_sb[:, n * 512:(n + 1) * 512],
                                         start=True, stop=(bias1_row is None))
                        if bias1_row is not None:
                            nc.tensor.matmul(pg[:], ones1x128[:],
                                             bias1_row[:, n * 512:(n + 1) * 512],
                                             start=False, stop=True)
                        sg = xT_pool.tile([128, 512], F32, tag="sg")
                        nc.scalar.copy(sg[:], pg[:])
                        nc.sync.dma_start(
                            gx1_hbm[m * 128:(m + 1) * 128, n * 512:(n + 1) * 512],
                            sg[:])

            # ---- phase S1: GRU layer 1 scan
            with ExitStack() as sctx:
                _emit_gru_scan(
                    nc, tc, sctx, T=T,
                    gx_view=gx1_hbm.rearrange("(b t) g -> t b g", b=B),
                    U_sb=U1_sb, brh_row=brh1,
                    h1T_view=h1T_hbm.rearrange("(k p) (t b) -> t p k b",
                                               p=128, b=B),
                    id_pad=id_pad, id16f=id16f, name="s1")

            # ---- phase G2: gx2 = h1 @ W2 (+bias row)
            with tc.tile_pool(name="h1T_in", bufs=4) as h1T_pool, \
                 tc.tile_pool(name="ps_g2", bufs=4, space="PSUM") as ps_g2:
                for m in range(MT):
                    lhs = []
                    for k in range(KT):
                        lt = h1T_pool.tile([128, 128], F32R, tag="h1T")
                        nc.sync.dma_start(
                            lt[:], h1T_hbm[k * 128:(k + 1) * 128,
                                           m * 128:(m + 1) * 128])
                        lhs.append(lt)
                    for n in range(3):
                        pg = ps_g2.tile([128, 512], F32, tag="pg2")
                        for k in range(KT):
                            nc.tensor.matmul(pg[:], lhs[k][:],
                                             W2_sb[:, k, n * 512:(n + 1) * 512],
                                             start=(k == 0),
                                             stop=(k == KT - 1 and bias2_row is None))
                        if bias2_row is not None:
                            nc.tensor.matmul(pg[:], ones1x128[:],
                                             bias2_row[:, n * 512:(n + 1) * 512],
                                             start=False, stop=True)
                        sg = h1T_pool.tile([128, 512], F32, tag="sg2")
                        nc.scalar.copy(sg[:], pg[:])
                        nc.sync.dma_start(
                            gx2_hbm[m * 128:(m + 1) * 128, n * 512:(n + 1) * 512],
                            sg[:])

            # ---- phase S2: GRU layer 2 scan
            with ExitStack() as sctx:
                h_fin = _emit_gru_scan(
                    nc, tc, sctx, T=T,
                    gx_view=gx2_hbm.rearrange("(t b) g -> t b g", b=B),
                    U_sb=U2_sb, brh_row=brh2, h1T_view=None,
                    id_pad=id_pad, id16f=id16f, name="s2")

                # ---- head: out = h2_last @ Wd + bd, as one DVE row-reduce
                with tc.tile_pool(name="head_sb", bufs=1) as head_sb:
                    WdB = head_sb.tile([B, U], F32)
                    nc.sync.dma_start(
                        WdB[:],
                        bass.AP(tensor=Wd_d.tensor, offset=Wd_d.offset,
                                ap=[[0, B], [1, U]]))
                    prod = head_sb.tile([B, U], F32)
                    nc.vector.tensor_mul(prod[:], h_fin[:], WdB[:])
                    acc = head_sb.tile([B, 1], F32)
                    nc.vector.tensor_reduce(acc[:], prod[:],
                                            axis=mybir.AxisListType.X,
                                            op=mybir.AluOpType.add)
                    res = head_sb.tile([B, 1], F32)
                    nc.scalar.activation(res[:], acc[:],
                                         mybir.ActivationFunctionType.Identity,
                                         bias=bd_sb[:])
                    nc.sync.dma_start(out_d[:], res[:])

    nc.compile()
    return nc


_CACHED = {}


def _get_nc(key, **kw):
    if key not in _CACHED:
        _CACHED[key] = build_bass(**kw)
    return _CACHED[key]


def kernel(x, W1, U1, bi1, br1, W2, U2, bi2, br2, Wd, bd):
    x = np.ascontiguousarray(x, dtype=np.float32)
    kw = dict(
        with_bi1=bool(np.any(bi1)), with_br1=bool(np.any(br1)),
        with_bi2=bool(np.any(bi2)), with_br2=bool(np.any(br2)),
    )
    nc = _get_nc(("full", T_FULL) + tuple(sorted(kw.items())), T=T_FULL, **kw)

    in_maps = []
    for c in range(N_CORES):
        in_maps.append({
            "x": np.ascontiguousarray(x[c * B:(c + 1) * B]),
            "W1": np.asarray(W1, np.float32), "U1": np.asarray(U1, np.float32),
            "bi1": np.asarray(bi1, np.float32), "br1": np.asarray(br1, np.float32),
            "W2": np.asarray(W2, np.float32), "U2": np.asarray(U2, np.float32),
            "bi2": np.asarray(bi2, np.float32), "br2": np.asarray(br2, np.float32),
            "Wd": np.asarray(Wd, np.float32), "bd": np.asarray(bd, np.float32),
        })
    res = run_bass_kernel_spmd(nc, in_maps, core_ids=list(range(N_CORES)))
    out = np.concatenate([res.results[c]["out"] for c in range(N_CORES)], axis=0)
    return out.astype(np.float32)



# revision 2
# speedup vs baseline: 3.5171x; 3.5171x over previous
"""Two-layer Keras-style GRU (reset_after=True, sigmoid/relu) + dense head
on 8 Trainium2 NeuronCores, data-parallel over batch (16 rows/core).

Transposed formulation: all per-step tensors live as [128 hidden-part,
4 ktile, 16 batch] so the recurrence matmuls are rec.T[g] = sum_k
U(k,g).T @ hT_k (stationary = U tile, moving = hT, N=16, bf16) and the
gate elementwise runs on 64-elem free dims instead of 512.  gx (input
projections) are computed chunk-wise as gxT = W.T @ xT / W2.T @ h1T and
injected into PSUM via identity matmuls, one accumulation epoch per
bank.  Layer-2 scan trails layer-1 by one 16-step chunk; projection
matmuls interleave into the scan stream to fill PE gaps.
"""
import sys

sys.path.insert(0, "/opt/trn_rl_repo")
sys.path.insert(0, "/opt/trn_rl_repo/concourse")

import numpy as np

import concourse.bass as bass
import concourse.bacc as bacc
import concourse.tile as tile
from concourse import mybir
from concourse.bass_utils import run_bass_kernel_spmd
from concourse.masks import make_identity

F32 = mybir.dt.float32
BF16 = mybir.dt.bfloat16
NPBF16 = mybir.dt.np(BF16)
SIG = mybir.ActivationFunctionType.Sigmoid

N_CORES = 8
B_TOT, T_FULL, F_IN, U = 128, 512, 128, 512
B = B_TOT // N_CORES          # 16 local batch
G3 = 3 * U                    # 1536 gate cols
KT = U // 128                 # 4 k-tiles
GT = G3 // 128                # 12 gate tiles
CH = 16                       # steps per chunk
BTC = CH * B                  # 256 bt-cols per chunk


def build_bass(T=T_FULL, with_bi1=False, with_br1=False, with_bi2=False,
               with_br2=False):
    nc = bacc.Bacc("TRN2", target_bir_lowering=False, debug=False,
                   enable_asserts=False, num_devices=N_CORES)
    NCH = T // CH

    xT_d = nc.dram_tensor("xTb", [F_IN, B * T], BF16, kind="ExternalInput").ap()
    W1_d = nc.dram_tensor("W1b", [F_IN, G3], BF16, kind="ExternalInput").ap()
    U1_d = nc.dram_tensor("U1b", [U, G3], BF16, kind="ExternalInput").ap()
    W2_d = nc.dram_tensor("W2b", [U, G3], BF16, kind="ExternalInput").ap()
    U2_d = nc.dram_tensor("U2b", [U, G3], BF16, kind="ExternalInput").ap()
    Wd_d = nc.dram_tensor("Wdb", [U, 1], BF16, kind="ExternalInput").ap()
    bi1_d = nc.dram_tensor("bi1b", [1, G3], BF16, kind="ExternalInput").ap()
    br1_d = nc.dram_tensor("br1b", [1, G3], BF16, kind="ExternalInput").ap()
    bi2_d = nc.dram_tensor("bi2b", [1, G3], BF16, kind="ExternalInput").ap()
    br2_d = nc.dram_tensor("br2b", [1, G3], BF16, kind="ExternalInput").ap()
    bd_d = nc.dram_tensor("bd", [1], F32, kind="ExternalInput").ap()
    out_d = nc.dram_tensor("out", [B, 1], F32, kind="ExternalOutput").ap()

    with tile.TileContext(nc) as tc:
        from contextlib import ExitStack
        with ExitStack() as ctx:
            const = ctx.enter_context(tc.tile_pool(name="const", bufs=1))
            gx1p = ctx.enter_context(tc.tile_pool(name="gx1", bufs=2))
            gx2p = ctx.enter_context(tc.tile_pool(name="gx2", bufs=2))
            gatep = ctx.enter_context(tc.tile_pool(name="gates", bufs=3))
            ps1p = ctx.enter_context(tc.tile_pool(name="ps1", bufs=2, space="PSUM"))
            ps2p = ctx.enter_context(tc.tile_pool(name="ps2", bufs=2, space="PSUM"))
            pspp = ctx.enter_context(tc.tile_pool(name="psp", bufs=2, space="PSUM"))

            # ---- constants / weights
            idf = const.tile([128, 128], F32)
            make_identity(nc, idf)
            I128 = const.tile([128, 128], BF16)
            nc.vector.tensor_copy(I128[:], idf[:])

            W1sb = const.tile([128, G3], BF16)
            nc.sync.dma_start(W1sb[:], W1_d[:])
            U1sb = const.tile([128, KT, G3], BF16)
            nc.sync.dma_start(U1sb[:], U1_d.rearrange("(k p) g -> p k g", p=128))
            W2sb = const.tile([128, KT, G3], BF16)
            nc.sync.dma_start(W2sb[:], W2_d.rearrange("(k p) g -> p k g", p=128))
            U2sb = const.tile([128, KT, G3], BF16)
            nc.sync.dma_start(U2sb[:], U2_d.rearrange("(k p) g -> p k g", p=128))
            Wdsb = const.tile([128, KT, 1], BF16)
            nc.sync.dma_start(Wdsb[:], Wd_d.rearrange("(k p) o -> p k o", p=128))
            bdsb = const.tile([1, 1], F32)
            nc.sync.dma_start(bdsb[:], bd_d[None, :])

            xsb = const.tile([128, B * T], BF16)
            nc.sync.dma_start(xsb[:], xT_d[:])

            ones16 = None
            ones256 = None
            if with_br1 or with_br2:
                o16f = const.tile([1, B], F32)
                nc.vector.memset(o16f, 1.0)
                ones16 = const.tile([1, B], BF16)
                nc.vector.tensor_copy(ones16[:], o16f[:])
            if with_bi1 or with_bi2:
                o256f = const.tile([1, BTC], F32)
                nc.vector.memset(o256f, 1.0)
                ones256 = const.tile([1, BTC], BF16)
                nc.vector.tensor_copy(ones256[:], o256f[:])

            def _row(d, flag, tag):
                if not flag:
                    return None
                t = const.tile([1, G3], BF16, tag=tag)
                nc.sync.dma_start(t[:], d[:])
                return t

            bi1r = _row(bi1_d, with_bi1, "bi1")
            br1r = _row(br1_d, with_br1, "br1")
            bi2r = _row(bi2_d, with_bi2, "bi2")
            br2r = _row(br2_d, with_br2, "br2")

            # ---- state
            h1T = const.tile([128, KT, T + 1, B], BF16)
            nc.vector.memset(h1T[:, :, 0, :], 0.0)
            h2T = const.tile([128, KT, 2, B], BF16)
            nc.vector.memset(h2T[:, :, 0, :], 0.0)

            # ---- helpers
            def proj_chunk(c, which):
                """gxT chunk tile [128, GT, BTC] bf16 for chunk c."""
                if which == 1:
                    pool, birow, tag = gx1p, bi1r, "g1"
                else:
                    pool, birow, tag = gx2p, bi2r, "g2"
                g = pool.tile([128, GT, BTC], BF16, tag=tag)
                for gi in range(GT):
                    pp = pspp.tile([128, 512], F32, tag="pp")
                    if which == 1:
                        nc.tensor.matmul(
                            pp[:, 0:BTC], W1sb[:, gi * 128:(gi + 1) * 128],
                            xsb[:, c * BTC:(c + 1) * BTC],
                            start=True, stop=(birow is None))
                    else:
                        for k in range(KT):
                            nc.tensor.matmul(
                                pp[:, 0:BTC], W2sb[:, k, gi * 128:(gi + 1) * 128],
                                h1T[:, k, 1 + c * CH:1 + (c + 1) * CH, :],
                                start=(k == 0),
                                stop=(k == KT - 1 and birow is None))
                    if birow is not None:
                        nc.tensor.matmul(
                            pp[:, 0:BTC], birow[:, gi * 128:(gi + 1) * 128],
                            ones256[:], start=False, stop=True)
                    if gi % 2 == 0:
                        nc.scalar.copy(g[:, gi, :], pp[:, 0:BTC])
                    else:
                        nc.vector.tensor_copy(g[:, gi, :], pp[:, 0:BTC])
                return g

            def scan_step(s, ps_pool, Usb, gxt, tl, h_prev, h_next, brrow):
                ps = ps_pool.tile([128, 512], F32, tag=f"ps{s}")
                Z = ps[:, 0:64].rearrange("p (j b) -> p j b", j=KT)
                R = ps[:, 64:128].rearrange("p (j b) -> p j b", j=KT)
                H = ps[:, 128:192].rearrange("p (j b) -> p j b", j=KT)
                gsl = gxt[:, :, tl * B:(tl + 1) * B]
                # one accumulation epoch per step for the whole bank:
                # only the very first matmul clears has_written.
                # injects (I128 stationary loaded once)
                for j in range(KT):
                    nc.tensor.matmul(Z[:, j, :], I128[:], gsl[:, j, :],
                                     start=(j == 0), stop=False)
                for j in range(KT):
                    nc.tensor.matmul(R[:, j, :], I128[:], gsl[:, 4 + j, :],
                                     start=False, stop=False)
                if brrow is not None:
                    for bank, base in ((Z, 0), (R, 4), (H, 8)):
                        for j in range(KT):
                            gi = base + j
                            nc.tensor.matmul(
                                bank[:, j, :],
                                brrow[:, gi * 128:(gi + 1) * 128],
                                ones16[:], start=False, stop=False)
                # recurrence: Z bank, then R, then H; stop on final matmul
                for bank, base in ((Z, 0), (R, 4), (H, 8)):
                    for j in range(KT):
                        gi = base + j
                        for k in range(KT):
                            last = (base == 8 and j == KT - 1 and k == KT - 1)
                            nc.tensor.matmul(
                                bank[:, j, :],
                                Usb[:, k, gi * 128:(gi + 1) * 128],
                                h_prev[:, k, :], start=False, stop=last)
                    if base == 0:
                        z_sb = gatep.tile([128, KT, B], BF16, tag=f"z{s}")
                        nc.scalar.activation(z_sb[:], Z, SIG)
                        w_sb = gatep.tile([128, KT, B], BF16, tag=f"w{s}")
                        nc.scalar.activation(w_sb[:], Z, SIG, scale=-1.0)
                        zh = gatep.tile([128, KT, B], BF16, tag=f"zh{s}")
                        nc.gpsimd.tensor_mul(zh[:], z_sb[:], h_prev[:])
                    elif base == 4:
                        r_sb = gatep.tile([128, KT, B], F32, tag=f"r{s}")
                        nc.scalar.activation(r_sb[:], R, SIG)
                # candidate: hh = relu(gx_h + r * rec_h)
                t1 = gatep.tile([128, KT, B], BF16, tag=f"t1{s}")
                nc.vector.tensor_mul(t1[:], r_sb[:], H)
                t2 = gatep.tile([128, KT, B], BF16, tag=f"t2{s}")
                nc.vector.tensor_add(t2[:], t1[:], gsl[:, 8:12, :])
                hh = gatep.tile([128, KT, B], BF16, tag=f"hh{s}")
                nc.vector.tensor_scalar_max(hh[:], t2[:], 0.0)
                # h_new = z*h + (1-z)*hh
                t3 = gatep.tile([128, KT, B], BF16, tag=f"t3{s}")
                nc.gpsimd.tensor_mul(t3[:], w_sb[:], hh[:])
                nc.vector.tensor_add(h_next[:], t3[:], zh[:])

            # ---- main pipeline: scan2 trails scan1 by one chunk
            for c in range(NCH + 1):
                if c < NCH:
                    g1 = proj_chunk(c, 1)
                if c >= 1:
                    g2 = proj_chunk(c - 1, 2)
                for tl in range(CH):
                    if c < NCH:
                        tg = c * CH + tl
                        scan_step(1, ps1p, U1sb, g1, tl,
                                  h1T[:, :, tg, :], h1T[:, :, tg + 1, :], br1r)
                    if c >= 1:
                        tg = (c - 1) * CH + tl
                        scan_step(2, ps2p, U2sb, g2, tl,
                                  h2T[:, :, tg % 2, :],
                                  h2T[:, :, (tg + 1) % 2, :], br2r)

            # ---- head: out = h2_last @ Wd + bd
            hp = pspp.tile([1, B], F32, tag="head")
            h_fin = h2T[:, :, T % 2, :]
            for k in range(KT):
                nc.tensor.matmul(hp[:], Wdsb[:, k, :], h_fin[:, k, :],
                                 start=(k == 0), stop=(k == KT - 1))
            res = const.tile([1, B], F32)
            nc.scalar.activation(res[:], hp[:],
                                 mybir.ActivationFunctionType.Identity,
                                 bias=bdsb[:])
            nc.sync.dma_start(out_d.rearrange("b o -> o b"), res[:])

    nc.compile()
    return nc


def prep_core_inputs(inputs, c):
    """Map the full-problem inputs to core c's dram tensors."""
    x = np.asarray(inputs["x"], np.float32)[c * B:(c + 1) * B]
    xT = np.ascontiguousarray(x.transpose(2, 1, 0).reshape(F_IN, B * T_FULL))
    m = {
        "xTb": xT.astype(NPBF16),
        "W1b": np.asarray(inputs["W1"], np.float32).astype(NPBF16),
        "U1b": np.asarray(inputs["U1"], np.float32).astype(NPBF16),
        "W2b": np.asarray(inputs["W2"], np.float32).astype(NPBF16),
        "U2b": np.asarray(inputs["U2"], np.float32).astype(NPBF16),
        "Wdb": np.asarray(inputs["Wd"], np.float32).astype(NPBF16),
        "bi1b": np.asarray(inputs["bi1"], np.float32).reshape(1, G3).astype(NPBF16),
        "br1b": np.asarray(inputs["br1"], np.float32).reshape(1, G3).astype(NPBF16),
        "bi2b": np.asarray(inputs["bi2"], np.float32).reshape(1, G3).astype(NPBF16),
        "br2b": np.asarray(inputs["br2"], np.float32).reshape(1, G3).astype(NPBF16),
        "bd": np.asarray(inputs["bd"], np.float32).reshape(1),
    }
    return m


_CACHED = {}


def _get_nc(key, **kw):
    if key not in _CACHED:
        _CACHED[key] = build_bass(**kw)
    return _CACHED[key]


def kernel(x, W1, U1, bi1, br1, W2, U2, bi2, br2, Wd, bd):
    inputs = dict(x=x, W1=W1, U1=U1, bi1=bi1, br1=br1, W2=W2, U2=U2,
                  bi2=bi2, br2=br2, Wd=Wd, bd=bd)
    kw = dict(
        with_bi1=bool(np.any(bi1)), with_br1=bool(np.any(br1)),
        with_bi2=bool(np.any(bi2)), with_br2=bool(np.any(br2)),
    )
    nc = _get_nc(("v2", T_FULL) + tuple(sorted(kw.items())), T=T_FULL, **kw)
    in_maps = [prep_core_inputs(inputs, c) for c in range(N_CORES)]
    res = run_bass_kernel_spmd(nc, in_maps, core_ids=list(range(N_CORES)))
    out = np.concatenate([res.results[c]["out"] for c in range(N_CORES)], axis=0)
    return out.astype(np.float32)
